# revision 15
# baseline (speedup 1.0000x reference)
# NonLocalBlock Trainium2 Bass kernel.
#
# Reference computation (per batch b):
#   theta = theta_w @ X + theta_b          [IC, N]   (X = x[b] as [C, N])
#   phi   = phi_w   @ X + phi_b            [IC, N]
#   g     = g_w     @ X + g_b              [IC, N]
#   attn  = softmax_j(theta^T phi)         [N, N]
#   att   = g @ attn^T                     [IC, N]
#   y     = BN(w_w @ att + w_b) + x
#
# Math folds used on device (validated vs reference):
#   - phi bias drops out of softmax entirely (adds an i-only constant).
#   - g bias folds into the final bias because attn rows sum to 1.
#   - BN is affine: fold into w_eff = inv*w_w and b_final.
#   - scores bounded (|s| < 50) so exp() needs no max-subtraction.
#
# Sharding: 8 cores = 4 batches x 2 row-halves. Each core receives x[b]
# with its own half's columns swapped to the front, so every core runs an
# identical program (pure SPMD): it projects theta for columns 0..2047
# ("own" rows i) and phi/g for all 4096 columns (keys/values j), computes
# 2048x4096 attention flash-style, and emits y for its own 2048 columns.
#
# Layout: scores are computed TRANSPOSED (j on partitions, i free) so the
# exp() output feeds att = g @ attn^T directly as lhsT, and the softmax
# denominator accumulates via an all-ones [128,128] stationary matmul into
# its own PSUM bank (result replicated across partitions, so the
# reciprocal+normalize need no cross-partition broadcast). All matmuls run
# as float32r (full PE rate at >=256 moving columns).
#
# Schedule notes (from HW traces):
#   - ~40 tiny warmup matmuls at t=0 get the PE HAM clock to 2.4 GHz
#     while the x DMAs stream in; a dummy exp preloads the ACT table.
#   - phase 1 pipelines projections per 512-column slice behind the DMAs
#     and interleaves the g-transposes so the PE never idles long enough
#     to re-throttle.
#   - phase 2 consumes exp output two groups behind the scores matmuls so
#     the PE has ~2.8us of independent work per 1.1us exp -> no stalls.
#   - block tails (reciprocal/normalize/W/store) are emitted one block
#     late so their latency hides under the next block's matmuls.

from contextlib import ExitStack

import numpy as np

import concourse.bass as bass
import concourse.tile as tile
from concourse import bacc, mybir
from concourse.bass_utils import run_bass_kernel_spmd
from concourse.masks import make_identity

F32 = mybir.dt.float32
F32R = mybir.dt.float32r
AF = mybir.ActivationFunctionType

B, C, IC = 4, 256, 128
H = W = 64
N = H * W            # 4096
HALF = N // 2        # 2048 rows of attention per core
P = 128
NCORES = 8
NBLK = HALF // 512   # 4 i-blocks of 512
NCH = N // P         # 32 j-chunks of 128
NGRP = NCH // 2      # 16 groups of 2 chunks per i-block
DEFER = 2            # consume exp output this many groups late
NWARM = 12           # HAM warmup matmuls at t=0 (512-col)
BN_EPS = 1e-5


def _r(ap):
    return ap.bitcast(F32R)


def _emit_consume(nc, pools, blk, grp):
    """AV + denominator matmuls for group `grp` of block `blk`."""
    att_ps = pools["att_ps"][blk]
    den_ps = pools["den_ps"][blk]
    gT_sb, onesP_sb = pools["gT_sb"], pools["onesP_sb"]
    ex_sb = pools["ex_sbs"][(blk, grp)]
    for c in range(2):
        jc = grp * 2 + c
        nc.tensor.matmul(
            den_ps[:], onesP_sb[:],
            ex_sb[:, c * 512:(c + 1) * 512],
            start=jc == 0, stop=jc == NCH - 1)
    for c in range(2):
        jc = grp * 2 + c
        nc.tensor.matmul(
            att_ps[:], gT_sb[:, jc * P:(jc + 1) * P],
            ex_sb[:, c * 512:(c + 1) * 512],
            start=jc == 0, stop=jc == NCH - 1)


def _emit_group(nc, pools, blk, grp):
    """Scores + exp for one [128,1024] group, consuming DEFER groups late."""
    ps_pool, ex_pool = pools["ps"], pools["ex"]
    theta_sb, phi_sb = pools["theta_sb"], pools["phi_sb"]
    isl = slice(blk * 512, (blk + 1) * 512)
    if grp == 0:
        pools["att_ps"][blk] = ps_pool.tile(
            [P, 512], F32, name=f"att_ps{blk}", tag="att", bufs=1)
        pools["den_ps"][blk] = ps_pool.tile(
            [P, 512], F32, name=f"den_ps{blk}", tag="den", bufs=1)
    sc_ps = ps_pool.tile([P, 1024], F32, name=f"sc{blk}_{grp}", tag="sc",
                         bufs=2)
    for c in range(2):
        jc = grp * 2 + c
        nc.tensor.matmul(
            sc_ps[:, c * 512:(c + 1) * 512],
            phi_sb[:, jc * P:(jc + 1) * P],
            theta_sb[:, isl],
            start=True, stop=True)
    ex_sb = ex_pool.tile([P, 1024], F32R, name=f"ex{blk}_{grp}", tag="ex")
    pools["ex_sbs"][(blk, grp)] = ex_sb
    nc.scalar.activation(ex_sb[:], sc_ps[:], AF.Exp)
    if grp >= DEFER:
        _emit_consume(nc, pools, blk, grp - DEFER)


def _emit_block_chunks(nc, pools, blk, skip=0):
    for grp in range(skip, NGRP):
        _emit_group(nc, pools, blk, grp)
    for grp in range(NGRP - DEFER, NGRP):
        _emit_consume(nc, pools, blk, grp)


def _emit_block_tail(nc, pools, blk, yout):
    """Softmax-normalize, W projection, bias+residual, store."""
    ps_pool, rec_pool = pools["ps"], pools["rec"]
    wef_sb, xb_sb = pools["wef_sb"], pools["xb_sb"]
    att_ps = pools["att_ps"][blk]
    den_ps = pools["den_ps"][blk]
    isl = slice(blk * 512, (blk + 1) * 512)

    rec_s = rec_pool.tile([P, 512], F32, name=f"rec_s{blk}", tag="rec_s")
    recb = rec_pool.tile([P, 512], F32, name=f"recb{blk}", tag="recb")
    nc.vector.reciprocal_approx_accurate(out=recb[:], in_=den_ps[:],
                                         scratch=rec_s[:])
    attn_sb = rec_pool.tile([P, 512], F32R, name=f"attn{blk}", tag="attn")
    nc.vector.tensor_mul(attn_sb[:], att_ps[:], recb[:])

    for k in range(2):
        y_ps = ps_pool.tile([P, 512], F32, name=f"y{blk}_{k}", tag="pp",
                            bufs=2)
        nc.tensor.matmul(
            y_ps[:], wef_sb[:, k * P:(k + 1) * P], attn_sb[:],
            start=True, stop=True)
        yo = rec_pool.tile([P, 512], F32, name=f"yo{blk}_{k}", tag="yo")
        nc.vector.tensor_add(yo[:], y_ps[:], xb_sb[k][:, isl])
        nc.sync.dma_start(out=yout[k * P:(k + 1) * P, isl], in_=yo[:])


def _kernel_body(ctx, tc, ins, yout):
    nc = tc.nc
    xin, thw, phw, gw, wef, tb, bfin = (
        ins["xin"], ins["thw"], ins["phw"], ins["gw"], ins["wef"],
        ins["tb"], ins["bfin"])

    consts = ctx.enter_context(tc.tile_pool(name="consts", bufs=1))
    big = ctx.enter_context(tc.tile_pool(name="big", bufs=1))

    # ---- dummy tiles for HAM warmup ----
    dum_f = consts.tile([P, 512], F32, name="dum_f")
    nc.vector.memset(dum_f[:], 1.0)
    dum_r = consts.tile([P, 512], F32R, name="dum_r")
    nc.vector.tensor_copy(dum_r[:], dum_f[:])

    # ---- x load: 512-col slices, alternating between both HWDGE rings.
    # Slice 0 + projection weights issue first; tail-only weights
    # (wef/tb/bfin) issue last.
    x_sb = [big.tile([P, N], F32R, name=f"x_sb{k}") for k in range(2)]

    def xdma(t, k):
        tsl = slice(t * 512, (t + 1) * 512)
        eng = nc.sync if (2 * t + k) % 2 == 0 else nc.scalar
        eng.dma_start(out=x_sb[k][:, tsl],
                      in_=_r(xin[k * P:(k + 1) * P, tsl]))

    for k in range(2):
        xdma(0, k)
    thw_sb = consts.tile([P, C], F32R, name="thw_sb")
    phw_sb = consts.tile([P, C], F32R, name="phw_sb")
    gw_sb = consts.tile([P, C], F32R, name="gw_sb")
    for k in range(2):
        nc.sync.dma_start(out=thw_sb[:, k * P:(k + 1) * P],
                          in_=_r(thw[k * P:(k + 1) * P, :]))
        nc.scalar.dma_start(out=phw_sb[:, k * P:(k + 1) * P],
                            in_=_r(phw[k * P:(k + 1) * P, :]))
        nc.sync.dma_start(out=gw_sb[:, k * P:(k + 1) * P],
                          in_=_r(gw[k * P:(k + 1) * P, :]))
    for t in range(1, 8):
        for k in range(2):
            xdma(t, k)
    exdum = consts.tile([P, 1], F32, name="exdum")
    nc.scalar.activation(exdum[:], dum_f[:, 0:1], AF.Exp)  # load exp table
    wef_sb = consts.tile([P, C], F32R, name="wef_sb")
    nc.sync.dma_start(out=wef_sb[:], in_=_r(wef[:, :]))
    tb_sb = consts.tile([P, 1], F32, name="tb_sb")
    nc.sync.dma_start(out=tb_sb[:], in_=tb[:, None])
    bfin_sb = consts.tile([P, 2], F32, name="bfin_sb")
    nc.sync.dma_start(out=bfin_sb[:], in_=bfin.rearrange("(k p) -> p k", p=P))
    onesP_f = consts.tile([P, P], F32, name="onesP_f")
    nc.vector.memset(onesP_f[:], 1.0)
    onesP_sb = consts.tile([P, P], F32R, name="onesP_sb")
    nc.vector.tensor_copy(onesP_sb[:], onesP_f[:])
    ident = consts.tile([P, P], F32, name="ident")
    make_identity(nc, ident[:])

    theta_sb = big.tile([P, HALF], F32R, name="theta_sb")
    phi_sb = big.tile([P, N], F32R, name="phi_sb")
    g_sb = big.tile([P, N], F32, name="g_sb")
    gT_sb = big.tile([P, N], F32R, name="gT_sb")
    xb_sb = [big.tile([P, HALF], F32, name=f"xb_sb{k}") for k in range(2)]

    # ---- single PSUM pool, tagged slots (8 banks total):
    #   sc 2x[128,1024]=4, att 1, den 1, pp 2x[128,512]=2 (proj/transpose/y)
    ps_pool = ctx.enter_context(tc.tile_pool(name="ps", bufs=1, space="PSUM"))
    pools = {
        "ps": ps_pool,
        "ex": ctx.enter_context(tc.tile_pool(name="ex", bufs=3 + DEFER)),
        "rec": ctx.enter_context(tc.tile_pool(name="rec", bufs=2)),
        "theta_sb": theta_sb, "phi_sb": phi_sb, "gT_sb": gT_sb,
        "onesP_sb": onesP_sb, "wef_sb": wef_sb, "xb_sb": xb_sb,
        "att_ps": {}, "den_ps": {}, "ex_sbs": {},
    }

    # ---- phase 1 (slice-pipelined projections + transposes) interleaved
    # with block 0 of the attention so the PE starts real work as soon as
    # the first x slice lands.
    dum_ps = ps_pool.tile([P, 512], F32, name="dum_ps", tag="pp", bufs=2)
    for i in range(NWARM):
        nc.tensor.matmul(dum_ps[:], dum_r[:, 0:P], dum_r[:],
                         start=True, stop=True)

    def transposes(t):
        for jc in range(4 * t, 4 * t + 4):
            jsl = slice(jc * P, (jc + 1) * P)
            pst = ps_pool.tile([P, P], F32, name=f"gt_ps{jc}", tag="pp",
                               bufs=2)
            nc.tensor.transpose(pst[:], g_sb[:, jsl], ident[:])
            nc.vector.tensor_copy(gT_sb[:, jsl], pst[:])

    def proj(t):
        tsl = slice(t * 512, (t + 1) * 512)
        if t < NBLK:
            ps = ps_pool.tile([P, 512], F32, name=f"th_ps{t}", tag="pp",
                              bufs=2)
            for k in range(2):
                nc.tensor.matmul(ps[:], thw_sb[:, k * P:(k + 1) * P],
                                 x_sb[k][:, tsl],
                                 start=(k == 0), stop=(k == 1))
            nc.vector.tensor_scalar_add(theta_sb[:, tsl], ps[:], tb_sb[:])
        for name, wsb, dst in (("ph", phw_sb, phi_sb), ("g", gw_sb, g_sb)):
            ps = ps_pool.tile([P, 512], F32, name=f"{name}_ps{t}", tag="pp",
                              bufs=2)
            for k in range(2):
                nc.tensor.matmul(ps[:], wsb[:, k * P:(k + 1) * P],
                                 x_sb[k][:, tsl],
                                 start=(k == 0), stop=(k == 1))
            nc.vector.tensor_copy(dst[:, tsl], ps[:])

    proj(0)
    for t in range(1, 8):
        proj(t)
        transposes(t - 1)
        for gg in (2 * (t - 1), 2 * (t - 1) + 1):
            _emit_group(nc, pools, 0, gg)
    transposes(7)
    for k in range(2):
        nc.vector.tensor_scalar_add(xb_sb[k][:], x_sb[k][:, 0:HALF],
                                    bfin_sb[:, k:k + 1])
    _emit_block_chunks(nc, pools, 0, skip=14)

    # ---- remaining i-blocks, tails software-pipelined one block late ----
    for blk in range(1, NBLK):
        _emit_block_chunks(nc, pools, blk)
        _emit_block_tail(nc, pools, blk - 1, yout)
    _emit_block_tail(nc, pools, NBLK - 1, yout)


_CACHE = {}


def _build():
    if "nc" in _CACHE:
        return _CACHE["nc"]
    nc = bacc.Bacc("TRN2", target_bir_lowering=False, debug=False,
                   enable_asserts=False, num_devices=1)
    ins = {
        "xin": nc.dram_tensor("xin", [C, N], F32, kind="ExternalInput").ap(),
        "thw": nc.dram_tensor("thw", [C, IC], F32, kind="ExternalInput").ap(),
        "phw": nc.dram_tensor("phw", [C, IC], F32, kind="ExternalInput").ap(),
        "gw": nc.dram_tensor("gw", [C, IC], F32, kind="ExternalInput").ap(),
        "wef": nc.dram_tensor("wef", [IC, C], F32, kind="ExternalInput").ap(),
        "tb": nc.dram_tensor("tb", [IC], F32, kind="ExternalInput").ap(),
        "bfin": nc.dram_tensor("bfin", [C], F32, kind="ExternalInput").ap(),
    }
    yout = nc.dram_tensor("yout", [C, HALF], F32, kind="ExternalOutput").ap()
    with tile.TileContext(nc) as tc:
        with ExitStack() as ctx:
            _kernel_body(ctx, tc, ins, yout)
    nc.compile()
    _CACHE["nc"] = nc
    return nc


def _host_prepare(inputs):
    """Host-side folds + per-core input maps."""
    ii = {k: np.ascontiguousarray(np.asarray(v, dtype=np.float32))
          for k, v in inputs.items()}
    inv = ii["bn_gamma"] / np.sqrt(ii["bn_var"] + BN_EPS)
    w_eff = ii["w_w"] * inv[:, None]                       # [C, IC]
    b_final = (w_eff @ ii["g_b"] + ii["w_b"] * inv
               + ii["bn_beta"] - ii["bn_mean"] * inv)      # [C]
    shared = {
        "thw": np.ascontiguousarray(ii["theta_w"].T),      # [C, IC]
        "phw": np.ascontiguousarray(ii["phi_w"].T),
        "gw": np.ascontiguousarray(ii["g_w"].T),
        "wef": np.ascontiguousarray(w_eff.T),              # [IC, C]
        "tb": ii["theta_b"],
        "bfin": np.ascontiguousarray(b_final),
    }
    x = ii["x"].reshape(B, C, N)
    in_maps = []
    for core in range(NCORES):
        b, h = divmod(core, 2)
        own = x[b][:, h * HALF:(h + 1) * HALF]
        oth = x[b][:, (1 - h) * HALF:(2 - h) * HALF]
        xin = np.ascontiguousarray(np.concatenate([own, oth], axis=1))
        in_maps.append({"xin": xin, **shared})
    return in_maps


def _gather(results, x_dtype):
    out = np.empty((B, C, N), dtype=np.float32)
    for core in range(NCORES):
        b, h = divmod(core, 2)
        out[b][:, h * HALF:(h + 1) * HALF] = results[core]["yout"]
    return out.reshape(B, C, H, W).astype(x_dtype, copy=False)


def kernel(**inputs):
    nc = _build()
    in_maps = _host_prepare(inputs)
    res = run_bass_kernel_spmd(nc, in_maps, core_ids=list(range(NCORES)))
    return _gather(res.results, np.asarray(inputs["x"]).dtype)
